# revision 1
# baseline (speedup 1.0000x reference)
"""Trainium2 Bass kernel for nn_Attention_49185965473844.

Math (per example b):
    q = x @ Wq ; k = x @ Wk ; v = x @ Wv          (x: [S, D], W*: [D, D], D=32)
    A[q,k]   = sum_s q[s,q] k[s,k]  = (Wq^T G Wk)[q,k],   G = x^T x   ([32, 32])
    scores   = softmax(A, axis=q)                 (normalize down columns)
    out[q,s] = sum_k scores[q,k] v[s,k] = (M @ x^T)[q,s], M = scores @ Wv^T

So the whole problem reduces to: one Gram matrix G = x^T x per example (the
only big contraction, streamed over S), a tiny 32x32 chain + softmax, and one
[32,32] @ [32,S] matmul against x^T (PE transposes of the resident x tile).

Sharding: pure data parallel over batch B=64 -> 8 examples per NeuronCore.
"""

import numpy as np

import concourse.bass as bass
import concourse.bacc as bacc
import concourse.tile as tile
from concourse import mybir
from concourse.bass_utils import run_bass_kernel_spmd

N_CORES = 8
B, S, D = 64, 8192, 32
PER_CORE = B // N_CORES  # 8

F32 = mybir.dt.float32
F32R = mybir.dt.float32r

# float32r (TF32-like reduced-precision PE matmul mode; 1 cyc/row at moving
# dim >= 256 vs fp32's 4, single-pass weight load) for the Gram, transpose
# and output matmuls. walrus requires f32r matmul inputs to come from
# producers with f32r output dtype: x/eye are declared f32r at the DRAM
# level (bits are plain fp32), and the M^T block-diagonal picks up f32r in
# its masking multiply. Measured end-to-end relative error ~3e-4.
USE_F32R = True


def build_nc(n_ex=PER_CORE, seq=S):
    """Build the per-core Bass program. Same program runs on all 8 cores.

    s-index decomposition: s = 128*c + p, chunk c = 4*t + j (quad t, partition
    block j), quad t = 4*g + h (store group g).  So
        s = 2048*g + 512*h + 128*j + p,   h in [0,4), j in [0,4), p in [0,128).
    The PE transpose of natural-tile quad t produces partition (j, d), free p;
    the final block-diagonal matmul then yields out rows (j, q) and free
    (h, p) per group — stored with one 3-dim DMA per (g, j), spread over the
    scalar/sync HWDGE rings and the gpsimd SWDGE ring.
    """
    assert seq % 2048 == 0
    n_chunks = seq // 128     # 128-row chunks of x
    n_quads = n_chunks // 4   # [128, 128] column blocks of the natural tile
    n_groups = n_quads // 4   # store groups: 4 quads -> [128, 512] out tiles

    nc = bacc.Bacc("TRN2", target_bir_lowering=False, debug=False)
    # x is declared float32r: bits are plain fp32; the PE's f32r matmuls
    # truncate mantissas internally (TF32-like). This satisfies the
    # birverifier's "rounded producer" rule via the dma's f32r output dtype.
    x_t = nc.declare_dram_parameter("x", [n_ex, seq, D], F32R, isOutput=False)
    eye_t = nc.declare_dram_parameter("eye", [128, 128], F32R, isOutput=False)
    cst_t = nc.declare_dram_parameter("cst", [128, 352], F32, isOutput=False)
    out_t = nc.declare_dram_parameter("out", [n_ex, D, seq], F32, isOutput=True)

    with tile.TileContext(nc) as tc:
        with (
            tc.tile_pool(name="consts", bufs=1) as consts,
            tc.tile_pool(name="nat_pool", bufs=n_ex) as nat_pool,
            tc.tile_pool(name="trhs_pool", bufs=4) as trhs_pool,
            tc.tile_pool(name="osb_pool", bufs=n_ex * n_groups) as osb_pool,
            tc.tile_pool(name="small_pool", bufs=3) as small_pool,
            tc.tile_pool(name="acc_psum", bufs=2, space="PSUM") as acc_psum,
            tc.tile_pool(name="tp_psum", bufs=1, space="PSUM") as tp_psum,
            tc.tile_pool(name="o_psum", bufs=2, space="PSUM") as o_psum,
        ):
            # ---- constants: one DMA so every PE consumer has a single
            # upstream sync (fp32 matmuls only get 1 sync wait in walrus) ----
            cst_sb = consts.tile([128, 352], F32)
            nc.sync.dma_start(out=cst_sb, in_=cst_t[:, :])
            identity = cst_sb[:, 0:128]
            wv4 = cst_sb[:, 128:160]       # np.tile(Wv, (4, 1))
            wq_sb = cst_sb[0:D, 160:192]
            wk_sb = cst_sb[0:D, 192:224]
            blkmask = cst_sb[:, 224:352]   # [p, c] = 1.0 iff p//32 == c//32
            # Wv replicated on 4 partition blocks, PE-transposed so that
            # wvt_rep[k, 32*j + d] = Wv[d, k].
            wvt_ps = acc_psum.tile([D, 128], F32, tag="acc")
            nc.tensor.transpose(wvt_ps, wv4, identity)
            wvt_rep = consts.tile([D, 128], F32)
            nc.scalar.copy(out=wvt_rep, in_=wvt_ps)
            ident_r = consts.tile([128, 128], F32R)
            nc.sync.dma_start(out=ident_r, in_=eye_t[:, :])

            def make_tp(nat2, g):
                """PE-transpose quads t = 4g + (0..3) into one PSUM bank."""
                tp_ps = tp_psum.tile(
                    [128, 512], F32, tag=f"tp{g}", bufs=1, name=f"tp_{g}"
                )
                for i in range(4):
                    t = 4 * g + i
                    nc.tensor.transpose(
                        tp_ps[:, 128 * i:128 * (i + 1)].bitcast(F32R),
                        nat2[:, 128 * t:128 * (t + 1)],
                        ident_r,
                    )
                return tp_ps

            def load_nat(b):
                # x_b as [128, n_chunks * 32]; chunk c col-block holds
                # x[128*c + p, :] on partition p
                nat = nat_pool.tile([128, n_chunks, D], F32R, tag="nat",
                                    name=f"nat_{b}")
                nc.sync.dma_start(
                    out=nat, in_=x_t[b].rearrange("(c p) d -> p c d", p=128)
                )
                return nat

            def emit_o_phase(b, tp_tiles, bd):
                """Output phase (trhs copies, block-diag matmuls, stores)
                for example b; copy engines alternate by group parity."""
                for g in range(n_groups):
                    trhs = trhs_pool.tile([128, 512], F32R if USE_F32R else F32,
                                          tag="trhs")
                    tp_src = tp_tiles[g].bitcast(F32R) if USE_F32R else tp_tiles[g]
                    if g % 2 == 0:
                        nc.scalar.copy(out=trhs, in_=tp_src)
                    else:
                        nc.vector.tensor_copy(out=trhs, in_=tp_src)
                    o_ps = o_psum.tile([128, 512], F32, tag="o")
                    nc.tensor.matmul(o_ps, lhsT=bd, rhs=trhs)
                    o_sb = osb_pool.tile([128, 512], F32, tag="o_sb")
                    if g % 2 == 0:
                        nc.scalar.copy(out=o_sb, in_=o_ps)
                    else:
                        nc.vector.tensor_copy(out=o_sb, in_=o_ps)
                    dst4 = out_t[b].rearrange(
                        "q (gg h j p) -> gg j q h p",
                        gg=n_groups, h=4, j=4, p=128,
                    )[g]
                    steng = nc.scalar if g % 2 == 0 else nc.sync
                    for j in range(4):
                        eng = steng if j < 2 else nc.gpsimd
                        eng.dma_start(
                            out=dst4[j], in_=o_sb[32 * j:32 * (j + 1), :]
                        )

            o_state = {}

            # All example loads are queued upfront (x is SBUF-resident for
            # the whole kernel): the DMA engines always have load packets
            # available, stores interleave at packet granularity, and no
            # load's descriptor generation ever queues behind store waits.
            nats = {b: load_nat(b) for b in range(n_ex)}
            for b in range(n_ex):
                nat = nats.pop(b)
                nat2 = nat.rearrange("p c d -> p (c d)")

                # ---- Gram accumulation: 128x128 of quad cross-products;
                # the 4 diagonal 32x32 blocks sum to G = x^T x ----
                # f32r runs 1 cyc/row only at moving dim >= 256: moving
                # block = quad pair, valid self-product in the left half.
                gram_ps = acc_psum.tile([128, 256], F32, tag="acc")
                for t in range(n_quads):
                    last = t == n_quads - 1
                    width = 128 if last else 256
                    nc.tensor.matmul(
                        gram_ps[:, 0:width],
                        lhsT=nat2[:, 128 * t:128 * (t + 1)],
                        rhs=nat2[:, 128 * t:128 * t + width],
                        start=(t == 0),
                        stop=last,
                        skip_group_check=True,
                    )

                # PE does group-0/1 transposes while ACT folds gram to SBUF.
                tp_tiles = {}
                tp_tiles[0] = make_tp(nat2, 0)
                tp_tiles[1] = make_tp(nat2, 1)

                # ---- fold the 4 diagonal 32x32 blocks of gram into G ----
                gram_sb = small_pool.tile([128, 128], F32, tag="gram_sb")
                nc.scalar.copy(out=gram_sb, in_=gram_ps[:, 0:128])
                g_ps = acc_psum.tile([D, D], F32, tag="acc")
                for j in range(4):
                    nc.tensor.matmul(
                        g_ps,
                        lhsT=identity[:, 32 * j:32 * (j + 1)],
                        rhs=gram_sb[:, 32 * j:32 * (j + 1)],
                        start=(j == 0),
                        stop=(j == 3),
                    )
                g_sb = small_pool.tile([D, D], F32, tag="g_sb")
                nc.scalar.copy(out=g_sb, in_=g_ps)

                # ---- A^T = Wk^T (G Wq);  G symmetric so lhsT=G works ----
                t2_ps = acc_psum.tile([D, D], F32, tag="acc")
                nc.tensor.matmul(t2_ps, lhsT=g_sb, rhs=wq_sb)
                t2_sb = small_pool.tile([D, D], F32, tag="t2_sb")
                nc.scalar.copy(out=t2_sb, in_=t2_ps)
                at_ps = acc_psum.tile([D, D], F32, tag="acc")
                nc.tensor.matmul(at_ps, lhsT=wk_sb, rhs=t2_sb)

                # ---- softmax over q (free dim of A^T), on DVE/ACT while the
                # PE runs the remaining transposes ----
                nmax = small_pool.tile([D, 1], F32, tag="nmax")
                nc.vector.reduce_max(
                    out=nmax, in_=at_ps, axis=mybir.AxisListType.X, negate=True
                )
                e_sb = small_pool.tile([D, D], F32, tag="e_sb")
                rsum = small_pool.tile([D, 1], F32, tag="rsum")
                # exp and its row-sum fused in one ACT instruction (accum_out)
                nc.scalar.activation(
                    out=e_sb, in_=at_ps,
                    func=mybir.ActivationFunctionType.Exp,
                    bias=nmax, scale=1.0,
                    accum_out=rsum,
                )
                rinv = small_pool.tile([D, 1], F32, tag="rinv")
                nc.vector.reciprocal(out=rinv, in_=rsum)
                sc_sb = small_pool.tile([D, D], F32, tag="sc_sb")
                nc.vector.tensor_scalar_mul(out=sc_sb, in0=e_sb, scalar1=rinv)

                for g in range(2, n_groups):
                    tp_tiles[g] = make_tp(nat2, g)

                # ---- M^T replicated on 4 partition blocks ----
                m4_ps = acc_psum.tile([128, D], F32, tag="acc")
                nc.tensor.matmul(m4_ps, lhsT=wvt_rep, rhs=sc_sb)
                m4_sb = small_pool.tile([128, D], F32, tag="m4_sb")
                nc.scalar.copy(out=m4_sb, in_=m4_ps)
                # Block-diagonal lhsT for the output matmuls: one full-width
                # matmul per group instead of four 32x32 sub-tile matmuls
                # (walrus rejects f32r + tile_position). The mask multiply
                # also performs the f32r rounding.
                bd = small_pool.tile([128, 128], F32R if USE_F32R else F32,
                                     tag="bd")
                m4_bcast = bass.AP(
                    tensor=m4_sb.tensor,
                    offset=m4_sb.offset,
                    ap=[list(m4_sb.ap[0]), [0, 4], list(m4_sb.ap[1])],
                )
                nc.gpsimd.tensor_mul(
                    out=bd.rearrange("p (r q) -> p r q", r=4),
                    in0=m4_bcast,
                    in1=blkmask.rearrange("p (r q) -> p r q", r=4),
                )

                emit_o_phase(b, tp_tiles, bd)

    nc.compile()
    return nc


_CACHED_NC = None


def _get_nc():
    global _CACHED_NC
    if _CACHED_NC is None:
        _CACHED_NC = build_nc()
    return _CACHED_NC


def make_cst(wq, wk, wv):
    """[128, 352]: identity | tile(Wv,(4,1)) | Wq | Wk | 32x32 block mask."""
    cst = np.zeros((128, 352), dtype=np.float32)
    cst[:, 0:128] = np.eye(128, dtype=np.float32)
    cst[:, 128:160] = np.tile(wv, (4, 1))
    cst[0:D, 160:192] = wq
    cst[0:D, 192:224] = wk
    blk = np.arange(128) // 32
    cst[:, 224:352] = (blk[:, None] == blk[None, :]).astype(np.float32)
    return cst


def kernel(x, Wq, Wk, Wv):
    x = np.ascontiguousarray(np.asarray(x, dtype=np.float32))
    wq = np.asarray(Wq, dtype=np.float32).reshape(D, D)
    wk = np.asarray(Wk, dtype=np.float32).reshape(D, D)
    wv = np.asarray(Wv, dtype=np.float32).reshape(D, D)
    assert x.shape == (B, S, D)
    cst = make_cst(wq, wk, wv)

    nc = _get_nc()
    eye = np.eye(128, dtype=np.float32)
    in_maps = [
        {
            "x": x[c * PER_CORE:(c + 1) * PER_CORE],
            "cst": cst,
            "eye": eye,
        }
        for c in range(N_CORES)
    ]
    res = run_bass_kernel_spmd(nc, in_maps, list(range(N_CORES)))
    out = np.concatenate([res.results[c]["out"] for c in range(N_CORES)], axis=0)
    return out



# revision 12
# speedup vs baseline: 1.5648x; 1.5648x over previous
"""Trainium2 Bass kernel for nn_Attention_49185965473844.

Math (per example b):
    q = x @ Wq ; k = x @ Wk ; v = x @ Wv          (x: [S, D], W*: [D, D], D=32)
    A[q,k]   = sum_s q[s,q] k[s,k]  = (Wq^T G Wk)[q,k],   G = x^T x   ([32, 32])
    scores   = softmax(A, axis=q)                 (normalize down columns)
    out[q,s] = sum_k scores[q,k] v[s,k] = (M @ x^T)[q,s], M = scores @ Wv^T

The kernel is DMA-bound (16.8 MB/core of fp32 I/O at ~358 GB/s/core).  Two
levers vs. the fp32 baseline:
  1. fp16 end-to-end (10-bit mantissa, same as the fp32r/TF32 PE mode the
     fp32 baseline used).  Measured end-to-end rel err ~8e-4 (limit 2e-2).
     Halves both load and store traffic.
  2. Host-side pre-transpose: x is uploaded as x^T tiles ([b, j, k, e] =
     x[b, 2048 j + e, k]) so the load, the output matmul rhs, and the store
     are all fully contiguous in HBM (4 KiB per partition line) -- the fp32
     baseline's 128 B load / 512 B store packets were the bottleneck.

On-chip per example: PE re-transposes x^T -> natural-layout tiles feeding the
Gram matmuls (G = x^T x accumulated over 16 column slabs + a diagonal fold),
the tiny 32x32 chain + softmax runs on ACT/DVE, and the output
out[(j,q), e] = sum_k M[q,k] x[2048 j + e, k] is 4 block-diagonal matmuls
against the resident x^T tile.  Stored as [b, j, q, e] fp16, un-permuted and
cast to fp32 on the host.

Sharding: pure data parallel over batch B=64 -> 8 examples per NeuronCore.
"""

import numpy as np

import concourse.bass as bass
import concourse.bacc as bacc
import concourse.tile as tile
from concourse import mybir
from concourse.bass_utils import run_bass_kernel_spmd

N_CORES = 8
B, S, D = 64, 8192, 32
PER_CORE = B // N_CORES  # 8

F32 = mybir.dt.float32
F16 = mybir.dt.float16

EB = S // 4  # 2048: e-range per j-block of the transposed layout


def build_nc(n_ex=PER_CORE, seq=S):
    """Build the per-core Bass program. Same program runs on all 8 cores.

    Layouts (all fp16):
      xt[b]  SBUF [128, 2048]: partition (j k), free e; xt[(j k), e] =
             x[2048 j + e, k].  HBM side is [b, j, k, e] -> fully contiguous.
      nat[b] SBUF [128, 2048]: partition p, free (t, j2, k);
             nat[p, (t j2 k)] = x[2048 j2 + 128 t + p, k]  (PE transposes of
             xt 128-col blocks).  Feeds the Gram matmuls.
      out[b] HBM [j, q, e] fp16 = out[q, 2048 j + e].
    """
    assert seq == S
    n_slab = seq // 512  # 16 transposes / Gram slabs per example

    nc = bacc.Bacc("TRN2", target_bir_lowering=False, debug=False)
    xt_t = nc.declare_dram_parameter("xt", [n_ex, 4, D, EB], F16, isOutput=False)
    c32_t = nc.declare_dram_parameter("c32", [128, 192], F32, isOutput=False)
    c16_t = nc.declare_dram_parameter("c16", [128, 256], F16, isOutput=False)
    out_t = nc.declare_dram_parameter("out", [n_ex, 4, D, EB], F16, isOutput=True)

    with tile.TileContext(nc) as tc:
        with (
            tc.tile_pool(name="consts", bufs=1) as consts,
            tc.tile_pool(name="xt_pool", bufs=n_ex) as xt_pool,
            tc.tile_pool(name="nat_pool", bufs=3) as nat_pool,
            tc.tile_pool(name="osb_pool", bufs=3) as osb_pool,
            tc.tile_pool(name="small_pool", bufs=3) as small_pool,
            tc.tile_pool(name="tp_psum", bufs=2, space="PSUM") as tp_psum,
            tc.tile_pool(name="acc_psum", bufs=2, space="PSUM") as acc_psum,
            tc.tile_pool(name="o_psum", bufs=2, space="PSUM") as o_psum,
        ):
            # ---- constants: one fp32 DMA + one fp16 DMA ----
            c32 = consts.tile([128, 192], F32)
            nc.sync.dma_start(out=c32, in_=c32_t[:, :])
            wq4 = c32[:, 0:32]           # np.tile(Wq, (4, 1))
            wk_sb = c32[0:D, 32:64]
            wvt_rep = c32[0:D, 64:192]   # wvt_rep[k, 32j + d] = Wv[d, k]
            c16 = consts.tile([128, 256], F16)
            nc.sync.dma_start(out=c16, in_=c16_t[:, :])
            eye16 = c16[:, 0:128]
            blkmask = c16[:, 128:256]  # [p, c] = 1.0 iff p//32 == c//32

            def load_xt(b):
                xt = xt_pool.tile([128, EB], F16, tag="xt", name=f"xt_{b}")
                nc.sync.dma_start(
                    out=xt, in_=xt_t[b].rearrange("j k e -> (j k) e")
                )
                return xt

            # gpsimd cannot read PSUM; PSUM->SBUF copies go to ACT/DVE only
            cp_engs = [nc.scalar, nc.vector, nc.scalar, nc.vector]

            def emit_a(b, xt):
                """Transposes + Gram + chain through softmax -> sc_sb."""
                nat = nat_pool.tile([128, 2048], F16, tag="nat", name=f"nat_{b}")

                def tp_group(g):
                    tp_ps = tp_psum.tile([128, 512], F16, tag="tp",
                                         name=f"tp_{b}_{g}")
                    for i in range(4):
                        m = 4 * g + i
                        nc.tensor.transpose(
                            tp_ps[:, 128 * i:128 * (i + 1)],
                            xt[:, 128 * m:128 * (m + 1)],
                            eye16,
                        )
                    eng = cp_engs[g]
                    if eng is nc.scalar:
                        eng.copy(out=nat[:, 512 * g:512 * (g + 1)], in_=tp_ps)
                    else:
                        eng.tensor_copy(out=nat[:, 512 * g:512 * (g + 1)],
                                        in_=tp_ps)

                def gram_quads(gram_ps, ts):
                    for t in ts:
                        last = t == n_slab - 1
                        width = 128 if last else 256
                        nc.tensor.matmul(
                            gram_ps[:, 0:width],
                            lhsT=nat[:, 128 * t:128 * (t + 1)],
                            rhs=nat[:, 128 * t:128 * t + width],
                            start=(t == 0),
                            stop=last,
                            skip_group_check=True,
                        )

                gram_ps = acc_psum.tile([128, 256], F32, tag="acc")
                tp_group(0)
                tp_group(1)
                tp_group(2)
                gram_quads(gram_ps, range(0, 4))
                tp_group(3)
                gram_quads(gram_ps, range(4, n_slab))

                # ---- fold: column-align the 4 diagonal 32x32 blocks with
                # partition-preserving copies; the j2-sum happens inside the
                # t2 matmul (contraction over all 128 partitions vs wq4) ----
                gram_c = small_pool.tile([128, D], F32, tag="gram_c")
                for j2 in range(4):
                    if j2 % 2 == 0:
                        nc.scalar.copy(
                            out=gram_c[32 * j2:32 * (j2 + 1), :],
                            in_=gram_ps[32 * j2:32 * (j2 + 1),
                                        32 * j2:32 * (j2 + 1)])
                    else:
                        nc.vector.tensor_copy(
                            out=gram_c[32 * j2:32 * (j2 + 1), :],
                            in_=gram_ps[32 * j2:32 * (j2 + 1),
                                        32 * j2:32 * (j2 + 1)])

                # ---- t2 = G Wq (fold + multiply);  A^T = Wk^T t2 ----
                t2_ps = acc_psum.tile([D, D], F32, tag="acc")
                nc.tensor.matmul(t2_ps, lhsT=gram_c, rhs=wq4)
                t2_sb = small_pool.tile([D, D], F32, tag="t2_sb")
                nc.scalar.copy(out=t2_sb, in_=t2_ps)
                at_ps = acc_psum.tile([D, D], F32, tag="acc")
                nc.tensor.matmul(at_ps, lhsT=wk_sb, rhs=t2_sb)

                # ---- softmax over q (free dim of A^T) on DVE/ACT ----
                nmax = small_pool.tile([D, 1], F32, tag="nmax")
                nc.vector.reduce_max(
                    out=nmax, in_=at_ps, axis=mybir.AxisListType.X, negate=True
                )
                e_sb = small_pool.tile([D, D], F32, tag="e_sb")
                rsum = small_pool.tile([D, 1], F32, tag="rsum")
                nc.scalar.activation(
                    out=e_sb, in_=at_ps,
                    func=mybir.ActivationFunctionType.Exp,
                    bias=nmax, scale=1.0,
                    accum_out=rsum,
                )
                rinv = small_pool.tile([D, 1], F32, tag="rinv")
                nc.vector.reciprocal(out=rinv, in_=rsum)
                sc_sb = small_pool.tile([D, D], F32, tag="sc_sb")
                nc.vector.tensor_scalar_mul(out=sc_sb, in0=e_sb, scalar1=rinv)
                return sc_sb

            def emit_m4bd(b, sc_sb):
                """M^T replicated on 4 partition blocks -> block-diag fp16."""
                m4_ps = acc_psum.tile([128, D], F32, tag="acc")
                nc.tensor.matmul(m4_ps, lhsT=wvt_rep, rhs=sc_sb)
                m4_sb = small_pool.tile([128, D], F16, tag="m4_sb")
                nc.scalar.copy(out=m4_sb, in_=m4_ps)
                bd = small_pool.tile([128, 128], F16, tag="bd")
                m4_bcast = bass.AP(
                    tensor=m4_sb.tensor,
                    offset=m4_sb.offset,
                    ap=[list(m4_sb.ap[0]), [0, 4], list(m4_sb.ap[1])],
                )
                nc.gpsimd.tensor_mul(
                    out=bd.rearrange("p (r q) -> p r q", r=4),
                    in0=m4_bcast,
                    in1=blkmask.rearrange("p (r q) -> p r q", r=4),
                )
                return bd

            def emit_out(b, xt, bd):
                """4 block-diagonal matmuls against resident x^T + store."""
                osb = osb_pool.tile([128, 2048], F16, tag="osb",
                                    name=f"osb_{b}")
                for h in range(2):
                    o_ps = o_psum.tile([128, 1024], F32, tag="o")
                    for g in range(2):
                        nc.tensor.matmul(
                            o_ps[:, 512 * g:512 * (g + 1)],
                            lhsT=bd,
                            rhs=xt[:, 1024 * h + 512 * g:1024 * h + 512 * (g + 1)],
                        )
                    eng = nc.scalar if h == 0 else nc.vector
                    if eng is nc.scalar:
                        eng.copy(out=osb[:, 1024 * h:1024 * (h + 1)], in_=o_ps)
                    else:
                        eng.tensor_copy(out=osb[:, 1024 * h:1024 * (h + 1)],
                                        in_=o_ps)
                nc.scalar.dma_start(
                    out=out_t[b].rearrange("j q e -> (j q) e"), in_=osb
                )

            # All loads queued upfront: the sync ring streams 4 MiB of
            # contiguous reads while compute pipelines behind it.
            xts = [load_xt(b) for b in range(n_ex)]
            # Depth-3 software pipeline: A(b) | m4bd(b-1) | out(b-2), so the
            # PE never waits on the cross-engine softmax/bd latency.
            scs = {}
            bds = {}
            for b in range(n_ex):
                scs[b] = emit_a(b, xts[b])
                if b >= 1:
                    bds[b - 1] = emit_m4bd(b - 1, scs.pop(b - 1))
                if b >= 2:
                    emit_out(b - 2, xts[b - 2], bds.pop(b - 2))
            bds[n_ex - 1] = emit_m4bd(n_ex - 1, scs.pop(n_ex - 1))
            emit_out(n_ex - 2, xts[n_ex - 2], bds.pop(n_ex - 2))
            emit_out(n_ex - 1, xts[n_ex - 1], bds.pop(n_ex - 1))

    nc.compile()
    return nc


_CACHED_NC = None


def _get_nc():
    global _CACHED_NC
    if _CACHED_NC is None:
        _CACHED_NC = build_nc()
    return _CACHED_NC


def make_consts(wq, wk, wv):
    """c32 [128, 192] fp32: Wq tiled 4x down | Wk | Wv^T tiled 4x along cols.
    c16 [128, 256] fp16: identity128 | 32x32 block mask."""
    c32 = np.zeros((128, 192), dtype=np.float32)
    c32[:, 0:32] = np.tile(wq, (4, 1))
    c32[0:D, 32:64] = wk
    c32[0:D, 64:192] = np.tile(wv.T, (1, 4))
    c16 = np.zeros((128, 256), dtype=np.float16)
    c16[:, 0:128] = np.eye(128, dtype=np.float16)
    blk = np.arange(128) // 32
    c16[:, 128:256] = (blk[:, None] == blk[None, :]).astype(np.float16)
    return c32, c16


def make_xt(x_slice):
    """[n, 8192, 32] fp32 -> [n, 4, 32, 2048] fp16 with
    xt[b, j, k, e] = x[b, 2048 j + e, k]."""
    n = x_slice.shape[0]
    return np.ascontiguousarray(
        x_slice.reshape(n, 4, EB, D).transpose(0, 1, 3, 2)
    ).astype(np.float16)


def unpack_out(res_out):
    """[n, 4, 32, 2048] fp16 -> [n, 32, 8192] fp32."""
    return np.ascontiguousarray(
        res_out.astype(np.float32).transpose(0, 2, 1, 3)
    ).reshape(res_out.shape[0], D, S)


def make_in_maps(x, wq, wk, wv):
    c32, c16 = make_consts(wq, wk, wv)
    return [
        {
            "xt": make_xt(x[c * PER_CORE:(c + 1) * PER_CORE]),
            "c32": c32,
            "c16": c16,
        }
        for c in range(N_CORES)
    ]


def kernel(x, Wq, Wk, Wv):
    x = np.asarray(x, dtype=np.float32)
    wq = np.asarray(Wq, dtype=np.float32).reshape(D, D)
    wk = np.asarray(Wk, dtype=np.float32).reshape(D, D)
    wv = np.asarray(Wv, dtype=np.float32).reshape(D, D)
    assert x.shape == (B, S, D)

    nc = _get_nc()
    in_maps = make_in_maps(x, wq, wk, wv)
    res = run_bass_kernel_spmd(nc, in_maps, list(range(N_CORES)))
    out = np.concatenate(
        [unpack_out(res.results[c]["out"]) for c in range(N_CORES)], axis=0
    )
    return out


# revision 13
# speedup vs baseline: 1.6634x; 1.0630x over previous
"""Trainium2 Bass kernel for nn_Attention_49185965473844.

Math (per example b):
    q = x @ Wq ; k = x @ Wk ; v = x @ Wv          (x: [S, D], W*: [D, D], D=32)
    A[q,k]   = sum_s q[s,q] k[s,k]  = (Wq^T G Wk)[q,k],   G = x^T x   ([32, 32])
    scores   = softmax(A, axis=q)                 (normalize down columns)
    out[q,s] = sum_k scores[q,k] v[s,k] = (M @ x^T)[q,s], M = scores @ Wv^T

The kernel is DMA-bound (16.8 MB/core of fp32 I/O at ~358 GB/s/core).  Two
levers vs. the fp32 baseline:
  1. fp16 end-to-end (10-bit mantissa, same as the fp32r/TF32 PE mode the
     fp32 baseline used).  Measured end-to-end rel err ~8e-4 (limit 2e-2).
     Halves both load and store traffic.
  2. Host-side pre-transpose: x is uploaded as x^T tiles ([b, j, k, e] =
     x[b, 2048 j + e, k]) so the load, the output matmul rhs, and the store
     are all fully contiguous in HBM (4 KiB per partition line) -- the fp32
     baseline's 128 B load / 512 B store packets were the bottleneck.

On-chip per example: PE re-transposes x^T -> natural-layout tiles feeding the
Gram matmuls (G = x^T x accumulated over 16 column slabs + a diagonal fold),
the tiny 32x32 chain + softmax runs on ACT/DVE, and the output
out[(j,q), e] = sum_k M[q,k] x[2048 j + e, k] is 4 block-diagonal matmuls
against the resident x^T tile.  Stored as [b, j, q, e] fp16, un-permuted and
cast to fp32 on the host.

Sharding: pure data parallel over batch B=64 -> 8 examples per NeuronCore.
"""

import numpy as np

import concourse.bass as bass
import concourse.bacc as bacc
import concourse.tile as tile
from concourse import mybir
from concourse.bass_utils import run_bass_kernel_spmd

N_CORES = 8
B, S, D = 64, 8192, 32
PER_CORE = B // N_CORES  # 8

F32 = mybir.dt.float32
F16 = mybir.dt.float16

EB = S // 4  # 2048: e-range per j-block of the transposed layout


def build_nc(n_ex=PER_CORE, seq=S):
    """Build the per-core Bass program. Same program runs on all 8 cores.

    Layouts (all fp16):
      xt[b]  SBUF [128, 2048]: partition (j k), free e; xt[(j k), e] =
             x[2048 j + e, k].  HBM side is [b, j, k, e] -> fully contiguous.
      nat[b] SBUF [128, 2048]: partition p, free (t, j2, k);
             nat[p, (t j2 k)] = x[2048 j2 + 128 t + p, k]  (PE transposes of
             xt 128-col blocks).  Feeds the Gram matmuls.
      out[b] HBM [j, q, e] fp16 = out[q, 2048 j + e].
    """
    assert seq == S
    n_slab = seq // 512  # 16 transposes / Gram slabs per example

    nc = bacc.Bacc("TRN2", target_bir_lowering=False, debug=False)
    xt_t = nc.declare_dram_parameter("xt", [n_ex, 4, D, EB], F16, isOutput=False)
    c32_t = nc.declare_dram_parameter("c32", [128, 192], F32, isOutput=False)
    c16_t = nc.declare_dram_parameter("c16", [128, 256], F16, isOutput=False)
    out_t = nc.declare_dram_parameter("out", [n_ex, 4, D, EB], F16, isOutput=True)

    with tile.TileContext(nc) as tc:
        with (
            tc.tile_pool(name="consts", bufs=1) as consts,
            tc.tile_pool(name="xt_pool", bufs=n_ex) as xt_pool,
            tc.tile_pool(name="nat_pool", bufs=3) as nat_pool,
            tc.tile_pool(name="osb_pool", bufs=3) as osb_pool,
            tc.tile_pool(name="small_pool", bufs=3) as small_pool,
            tc.tile_pool(name="tp_psum", bufs=2, space="PSUM") as tp_psum,
            tc.tile_pool(name="acc_psum", bufs=2, space="PSUM") as acc_psum,
            tc.tile_pool(name="o_psum", bufs=2, space="PSUM") as o_psum,
        ):
            # ---- constants: one fp32 DMA + one fp16 DMA ----
            c32 = consts.tile([128, 192], F32)
            nc.sync.dma_start(out=c32, in_=c32_t[:, :])
            wq4 = c32[:, 0:32]           # np.tile(Wq, (4, 1))
            wk_sb = c32[0:D, 32:64]
            wvt_rep = c32[0:D, 64:192]   # wvt_rep[k, 32j + d] = Wv[d, k]
            c16 = consts.tile([128, 256], F16)
            nc.sync.dma_start(out=c16, in_=c16_t[:, :])
            eye16 = c16[:, 0:128]
            blkmask = c16[:, 128:256]  # [p, c] = 1.0 iff p//32 == c//32

            def load_xt(b):
                xt = xt_pool.tile([128, EB], F16, tag="xt", name=f"xt_{b}")
                nc.sync.dma_start(
                    out=xt, in_=xt_t[b].rearrange("j k e -> (j k) e")
                )
                return xt

            # gpsimd cannot read PSUM; PSUM->SBUF copies go to ACT/DVE only
            cp_engs = [nc.scalar, nc.vector, nc.scalar, nc.vector]

            def emit_a(b, xt):
                """Transposes + Gram + chain through softmax -> sc_sb."""
                nat = nat_pool.tile([128, 2048], F16, tag="nat", name=f"nat_{b}")

                def tp_group(g):
                    tp_ps = tp_psum.tile([128, 512], F16, tag="tp",
                                         name=f"tp_{b}_{g}")
                    for i in range(4):
                        m = 4 * g + i
                        nc.tensor.transpose(
                            tp_ps[:, 128 * i:128 * (i + 1)],
                            xt[:, 128 * m:128 * (m + 1)],
                            eye16,
                        )
                    eng = cp_engs[g]
                    if eng is nc.scalar:
                        eng.copy(out=nat[:, 512 * g:512 * (g + 1)], in_=tp_ps)
                    else:
                        eng.tensor_copy(out=nat[:, 512 * g:512 * (g + 1)],
                                        in_=tp_ps)

                def gram_quads(gram_ps, ts):
                    # 128-wide rhs: fp16 streams 1 col-cycle/row, so wider
                    # tiles only add stream time (the >=256 moving-dim rule
                    # was f32r-specific).  The 4 diagonal 32x32 blocks of the
                    # [128,128] output are the per-j2 partial Grams.
                    for t in ts:
                        nc.tensor.matmul(
                            gram_ps,
                            lhsT=nat[:, 128 * t:128 * (t + 1)],
                            rhs=nat[:, 128 * t:128 * (t + 1)],
                            start=(t == 0),
                            stop=(t == n_slab - 1),
                        )

                gram_ps = acc_psum.tile([128, 128], F32, tag="acc")
                tp_group(0)
                tp_group(1)
                tp_group(2)
                gram_quads(gram_ps, range(0, 4))
                tp_group(3)
                gram_quads(gram_ps, range(4, n_slab))

                # ---- fold: column-align the 4 diagonal 32x32 blocks with
                # partition-preserving copies; the j2-sum happens inside the
                # t2 matmul (contraction over all 128 partitions vs wq4) ----
                gram_c = small_pool.tile([128, D], F32, tag="gram_c")
                for j2 in range(4):
                    if j2 % 2 == 0:
                        nc.scalar.copy(
                            out=gram_c[32 * j2:32 * (j2 + 1), :],
                            in_=gram_ps[32 * j2:32 * (j2 + 1),
                                        32 * j2:32 * (j2 + 1)])
                    else:
                        nc.vector.tensor_copy(
                            out=gram_c[32 * j2:32 * (j2 + 1), :],
                            in_=gram_ps[32 * j2:32 * (j2 + 1),
                                        32 * j2:32 * (j2 + 1)])

                # ---- t2 = G Wq (fold + multiply);  A^T = Wk^T t2 ----
                t2_ps = acc_psum.tile([D, D], F32, tag="acc")
                nc.tensor.matmul(t2_ps, lhsT=gram_c, rhs=wq4)
                t2_sb = small_pool.tile([D, D], F32, tag="t2_sb")
                nc.scalar.copy(out=t2_sb, in_=t2_ps)
                at_ps = acc_psum.tile([D, D], F32, tag="acc")
                nc.tensor.matmul(at_ps, lhsT=wk_sb, rhs=t2_sb)

                # ---- softmax over q (free dim of A^T) on DVE/ACT ----
                nmax = small_pool.tile([D, 1], F32, tag="nmax")
                nc.vector.reduce_max(
                    out=nmax, in_=at_ps, axis=mybir.AxisListType.X, negate=True
                )
                e_sb = small_pool.tile([D, D], F32, tag="e_sb")
                rsum = small_pool.tile([D, 1], F32, tag="rsum")
                nc.scalar.activation(
                    out=e_sb, in_=at_ps,
                    func=mybir.ActivationFunctionType.Exp,
                    bias=nmax, scale=1.0,
                    accum_out=rsum,
                )
                rinv = small_pool.tile([D, 1], F32, tag="rinv")
                nc.vector.reciprocal(out=rinv, in_=rsum)
                sc_sb = small_pool.tile([D, D], F32, tag="sc_sb")
                nc.vector.tensor_scalar_mul(out=sc_sb, in0=e_sb, scalar1=rinv)
                return sc_sb

            def emit_m4bd(b, sc_sb):
                """M^T replicated on 4 partition blocks -> block-diag fp16."""
                m4_ps = acc_psum.tile([128, D], F32, tag="acc")
                nc.tensor.matmul(m4_ps, lhsT=wvt_rep, rhs=sc_sb)
                m4_sb = small_pool.tile([128, D], F16, tag="m4_sb")
                nc.scalar.copy(out=m4_sb, in_=m4_ps)
                bd = small_pool.tile([128, 128], F16, tag="bd")
                m4_bcast = bass.AP(
                    tensor=m4_sb.tensor,
                    offset=m4_sb.offset,
                    ap=[list(m4_sb.ap[0]), [0, 4], list(m4_sb.ap[1])],
                )
                nc.gpsimd.tensor_mul(
                    out=bd.rearrange("p (r q) -> p r q", r=4),
                    in0=m4_bcast,
                    in1=blkmask.rearrange("p (r q) -> p r q", r=4),
                )
                return bd

            def emit_out(b, xt, bd):
                """4 block-diagonal matmuls against resident x^T + store."""
                osb = osb_pool.tile([128, 2048], F16, tag="osb",
                                    name=f"osb_{b}")
                for h in range(2):
                    o_ps = o_psum.tile([128, 1024], F32, tag="o")
                    for g in range(2):
                        nc.tensor.matmul(
                            o_ps[:, 512 * g:512 * (g + 1)],
                            lhsT=bd,
                            rhs=xt[:, 1024 * h + 512 * g:1024 * h + 512 * (g + 1)],
                        )
                    eng = nc.scalar if h == 0 else nc.vector
                    if eng is nc.scalar:
                        eng.copy(out=osb[:, 1024 * h:1024 * (h + 1)], in_=o_ps)
                    else:
                        eng.tensor_copy(out=osb[:, 1024 * h:1024 * (h + 1)],
                                        in_=o_ps)
                nc.scalar.dma_start(
                    out=out_t[b].rearrange("j q e -> (j q) e"), in_=osb
                )

            # All loads queued upfront: the sync ring streams 4 MiB of
            # contiguous reads while compute pipelines behind it.
            xts = [load_xt(b) for b in range(n_ex)]
            # Depth-3 software pipeline: A(b) | m4bd(b-1) | out(b-2), so the
            # PE never waits on the cross-engine softmax/bd latency.
            scs = {}
            bds = {}
            for b in range(n_ex):
                scs[b] = emit_a(b, xts[b])
                if b >= 1:
                    bds[b - 1] = emit_m4bd(b - 1, scs.pop(b - 1))
                if b >= 2:
                    emit_out(b - 2, xts[b - 2], bds.pop(b - 2))
            bds[n_ex - 1] = emit_m4bd(n_ex - 1, scs.pop(n_ex - 1))
            emit_out(n_ex - 2, xts[n_ex - 2], bds.pop(n_ex - 2))
            emit_out(n_ex - 1, xts[n_ex - 1], bds.pop(n_ex - 1))

    nc.compile()
    return nc


_CACHED_NC = None


def _get_nc():
    global _CACHED_NC
    if _CACHED_NC is None:
        _CACHED_NC = build_nc()
    return _CACHED_NC


def make_consts(wq, wk, wv):
    """c32 [128, 192] fp32: Wq tiled 4x down | Wk | Wv^T tiled 4x along cols.
    c16 [128, 256] fp16: identity128 | 32x32 block mask."""
    c32 = np.zeros((128, 192), dtype=np.float32)
    c32[:, 0:32] = np.tile(wq, (4, 1))
    c32[0:D, 32:64] = wk
    c32[0:D, 64:192] = np.tile(wv.T, (1, 4))
    c16 = np.zeros((128, 256), dtype=np.float16)
    c16[:, 0:128] = np.eye(128, dtype=np.float16)
    blk = np.arange(128) // 32
    c16[:, 128:256] = (blk[:, None] == blk[None, :]).astype(np.float16)
    return c32, c16


def make_xt(x_slice):
    """[n, 8192, 32] fp32 -> [n, 4, 32, 2048] fp16 with
    xt[b, j, k, e] = x[b, 2048 j + e, k]."""
    n = x_slice.shape[0]
    return np.ascontiguousarray(
        x_slice.reshape(n, 4, EB, D).transpose(0, 1, 3, 2)
    ).astype(np.float16)


def unpack_out(res_out):
    """[n, 4, 32, 2048] fp16 -> [n, 32, 8192] fp32."""
    return np.ascontiguousarray(
        res_out.astype(np.float32).transpose(0, 2, 1, 3)
    ).reshape(res_out.shape[0], D, S)


def make_in_maps(x, wq, wk, wv):
    c32, c16 = make_consts(wq, wk, wv)
    return [
        {
            "xt": make_xt(x[c * PER_CORE:(c + 1) * PER_CORE]),
            "c32": c32,
            "c16": c16,
        }
        for c in range(N_CORES)
    ]


def kernel(x, Wq, Wk, Wv):
    x = np.asarray(x, dtype=np.float32)
    wq = np.asarray(Wq, dtype=np.float32).reshape(D, D)
    wk = np.asarray(Wk, dtype=np.float32).reshape(D, D)
    wv = np.asarray(Wv, dtype=np.float32).reshape(D, D)
    assert x.shape == (B, S, D)

    nc = _get_nc()
    in_maps = make_in_maps(x, wq, wk, wv)
    res = run_bass_kernel_spmd(nc, in_maps, list(range(N_CORES)))
    out = np.concatenate(
        [unpack_out(res.results[c]["out"]) for c in range(N_CORES)], axis=0
    )
    return out


# revision 14
# speedup vs baseline: 1.7623x; 1.0594x over previous
"""Trainium2 Bass kernel for nn_Attention_49185965473844.

Math (per example b):
    q = x @ Wq ; k = x @ Wk ; v = x @ Wv          (x: [S, D], W*: [D, D], D=32)
    A[q,k]   = sum_s q[s,q] k[s,k]  = (Wq^T G Wk)[q,k],   G = x^T x   ([32, 32])
    scores   = softmax(A, axis=q)                 (normalize down columns)
    out[q,s] = sum_k scores[q,k] v[s,k] = (M @ x^T)[q,s], M = scores @ Wv^T

The kernel is DMA- and PE-bound.  Three levers vs. the fp32 baseline:
  1. fp16 end-to-end (10-bit mantissa, same as the f32r/TF32 PE mode the
     fp32 baseline used).  Measured end-to-end rel err ~8e-4 (limit 2e-2).
  2. Host-side pre-permutation of x into BOTH layouts the PE needs:
       xt[b,j,k,e]    = x[b, 2048 j + e, k]   (x^T tiles: output-matmul rhs)
       xn[b,p,t,j2,k] = x[b, 2048 j2 + 128 t + p, k]  (natural tiles: Gram)
     so there are NO on-chip transposes at all, and every DMA (loads and
     stores) is fully contiguous in HBM with 4 KiB per partition line.
  3. The tiny 32x32 chain runs on ACT/DVE with the j2-block fold done by
     partition-preserving copies + a 128-partition contraction (the PE
     cannot matmul at base partition 96).

Per example: 16 Gram matmuls (128x128 fp16, FWL weight loads), the 32x32
chain + softmax, 4 block-diagonal output matmuls [128,512] against the
resident x^T tile, one contiguous 512 KiB store ([b, j, q, e] fp16,
un-permuted and cast to fp32 on the host).

Sharding: pure data parallel over batch B=64 -> 8 examples per NeuronCore.
"""

import numpy as np

import concourse.bass as bass
import concourse.bacc as bacc
import concourse.tile as tile
from concourse import mybir
from concourse.bass_utils import run_bass_kernel_spmd

N_CORES = 8
B, S, D = 64, 8192, 32
PER_CORE = B // N_CORES  # 8

F32 = mybir.dt.float32
F16 = mybir.dt.float16

EB = S // 4  # 2048: e-range per j-block of the transposed layout


def build_nc(n_ex=PER_CORE, seq=S):
    """Build the per-core Bass program. Same program runs on all 8 cores."""
    assert seq == S
    n_slab = seq // 512  # 16 Gram slabs per example

    nc = bacc.Bacc("TRN2", target_bir_lowering=False, debug=False)
    xt_t = nc.declare_dram_parameter("xt", [n_ex, 4, D, EB], F16, isOutput=False)
    xn_t = nc.declare_dram_parameter("xn", [n_ex, 128, 2048], F16, isOutput=False)
    c32_t = nc.declare_dram_parameter("c32", [128, 192], F32, isOutput=False)
    c16_t = nc.declare_dram_parameter("c16", [128, 128], F16, isOutput=False)
    out_t = nc.declare_dram_parameter("out", [n_ex, 4, D, EB], F16, isOutput=True)

    with tile.TileContext(nc) as tc:
        with (
            tc.tile_pool(name="consts", bufs=1) as consts,
            tc.tile_pool(name="xt_pool", bufs=n_ex) as xt_pool,
            tc.tile_pool(name="xn_pool", bufs=n_ex) as xn_pool,
            tc.tile_pool(name="osb_pool", bufs=3) as osb_pool,
            tc.tile_pool(name="small_pool", bufs=3) as small_pool,
            tc.tile_pool(name="acc_psum", bufs=3, space="PSUM") as acc_psum,
            tc.tile_pool(name="o_psum", bufs=2, space="PSUM") as o_psum,
        ):
            # ---- constants: one fp32 DMA + one fp16 DMA ----
            c32 = consts.tile([128, 192], F32)
            nc.sync.dma_start(out=c32, in_=c32_t[:, :])
            wq4 = c32[:, 0:32]           # np.tile(Wq, (4, 1))
            wk_sb = c32[0:D, 32:64]
            wvt_rep = c32[0:D, 64:192]   # wvt_rep[k, 32j + d] = Wv[d, k]
            c16 = consts.tile([128, 128], F16)
            nc.sync.dma_start(out=c16, in_=c16_t[:, :])
            blkmask = c16[:, 0:128]      # [p, c] = 1.0 iff p//32 == c//32

            def load_pair(b):
                xn = xn_pool.tile([128, 2048], F16, tag="xn", name=f"xn_{b}")
                nc.sync.dma_start(out=xn, in_=xn_t[b])
                xt = xt_pool.tile([128, EB], F16, tag="xt", name=f"xt_{b}")
                nc.sync.dma_start(
                    out=xt, in_=xt_t[b].rearrange("j k e -> (j k) e")
                )
                return xn, xt

            def emit_a(b, xn):
                """Gram + chain through softmax -> sc_sb."""
                gram_ps = acc_psum.tile([128, 128], F32, tag="acc")
                for t in range(n_slab):
                    nc.tensor.matmul(
                        gram_ps,
                        lhsT=xn[:, 128 * t:128 * (t + 1)],
                        rhs=xn[:, 128 * t:128 * (t + 1)],
                        start=(t == 0),
                        stop=(t == n_slab - 1),
                    )

                # ---- fold: column-align the 4 diagonal 32x32 blocks with
                # partition-preserving copies; the j2-sum happens inside the
                # t2 matmul (contraction over all 128 partitions vs wq4) ----
                gram_c = small_pool.tile([128, D], F32, tag="gram_c")
                for j2 in range(4):
                    if j2 % 2 == 0:
                        nc.scalar.copy(
                            out=gram_c[32 * j2:32 * (j2 + 1), :],
                            in_=gram_ps[32 * j2:32 * (j2 + 1),
                                        32 * j2:32 * (j2 + 1)])
                    else:
                        nc.vector.tensor_copy(
                            out=gram_c[32 * j2:32 * (j2 + 1), :],
                            in_=gram_ps[32 * j2:32 * (j2 + 1),
                                        32 * j2:32 * (j2 + 1)])

                # ---- t2 = G Wq (fold + multiply);  A^T = Wk^T t2 ----
                t2_ps = acc_psum.tile([D, D], F32, tag="acc")
                nc.tensor.matmul(t2_ps, lhsT=gram_c, rhs=wq4)
                t2_sb = small_pool.tile([D, D], F32, tag="t2_sb")
                nc.scalar.copy(out=t2_sb, in_=t2_ps)
                at_ps = acc_psum.tile([D, D], F32, tag="acc")
                nc.tensor.matmul(at_ps, lhsT=wk_sb, rhs=t2_sb)

                # ---- softmax over q (free dim of A^T) on DVE/ACT ----
                nmax = small_pool.tile([D, 1], F32, tag="nmax")
                nc.vector.reduce_max(
                    out=nmax, in_=at_ps, axis=mybir.AxisListType.X, negate=True
                )
                e_sb = small_pool.tile([D, D], F32, tag="e_sb")
                rsum = small_pool.tile([D, 1], F32, tag="rsum")
                nc.scalar.activation(
                    out=e_sb, in_=at_ps,
                    func=mybir.ActivationFunctionType.Exp,
                    bias=nmax, scale=1.0,
                    accum_out=rsum,
                )
                rinv = small_pool.tile([D, 1], F32, tag="rinv")
                nc.vector.reciprocal(out=rinv, in_=rsum)
                sc_sb = small_pool.tile([D, D], F32, tag="sc_sb")
                nc.vector.tensor_scalar_mul(out=sc_sb, in0=e_sb, scalar1=rinv)
                return sc_sb

            def emit_m4bd(b, sc_sb):
                """M^T replicated on 4 partition blocks -> block-diag fp16."""
                m4_ps = acc_psum.tile([128, D], F32, tag="acc")
                nc.tensor.matmul(m4_ps, lhsT=wvt_rep, rhs=sc_sb)
                m4_sb = small_pool.tile([128, D], F16, tag="m4_sb")
                nc.scalar.copy(out=m4_sb, in_=m4_ps)
                bd = small_pool.tile([128, 128], F16, tag="bd")
                m4_bcast = bass.AP(
                    tensor=m4_sb.tensor,
                    offset=m4_sb.offset,
                    ap=[list(m4_sb.ap[0]), [0, 4], list(m4_sb.ap[1])],
                )
                nc.gpsimd.tensor_mul(
                    out=bd.rearrange("p (r q) -> p r q", r=4),
                    in0=m4_bcast,
                    in1=blkmask.rearrange("p (r q) -> p r q", r=4),
                )
                return bd

            def emit_out(b, xt, bd):
                """4 block-diagonal matmuls against resident x^T + store."""
                osb = osb_pool.tile([128, 2048], F16, tag="osb",
                                    name=f"osb_{b}")
                for h in range(2):
                    o_ps = o_psum.tile([128, 1024], F32, tag="o")
                    for g in range(2):
                        nc.tensor.matmul(
                            o_ps[:, 512 * g:512 * (g + 1)],
                            lhsT=bd,
                            rhs=xt[:, 1024 * h + 512 * g:1024 * h + 512 * (g + 1)],
                        )
                    if h == 0:
                        nc.scalar.copy(out=osb[:, 0:1024], in_=o_ps)
                    else:
                        nc.vector.tensor_copy(out=osb[:, 1024:2048], in_=o_ps)
                nc.scalar.dma_start(
                    out=out_t[b].rearrange("j q e -> (j q) e"), in_=osb
                )

            # All loads queued upfront: the sync ring streams 8 MiB of
            # contiguous reads while compute pipelines behind it.
            tiles = [load_pair(b) for b in range(n_ex)]
            # Depth-3 software pipeline: A(b) | m4bd(b-1) | out(b-2), so the
            # PE never waits on the cross-engine softmax/bd latency.
            scs = {}
            bds = {}
            for b in range(n_ex):
                scs[b] = emit_a(b, tiles[b][0])
                if b >= 1:
                    bds[b - 1] = emit_m4bd(b - 1, scs.pop(b - 1))
                if b >= 2:
                    emit_out(b - 2, tiles[b - 2][1], bds.pop(b - 2))
            bds[n_ex - 1] = emit_m4bd(n_ex - 1, scs.pop(n_ex - 1))
            emit_out(n_ex - 2, tiles[n_ex - 2][1], bds.pop(n_ex - 2))
            emit_out(n_ex - 1, tiles[n_ex - 1][1], bds.pop(n_ex - 1))

    nc.compile()
    return nc


_CACHED_NC = None


def _get_nc():
    global _CACHED_NC
    if _CACHED_NC is None:
        _CACHED_NC = build_nc()
    return _CACHED_NC


def make_consts(wq, wk, wv):
    """c32 [128, 192] fp32: Wq tiled 4x down | Wk | Wv^T tiled 4x along cols.
    c16 [128, 128] fp16: 32x32 block mask."""
    c32 = np.zeros((128, 192), dtype=np.float32)
    c32[:, 0:32] = np.tile(wq, (4, 1))
    c32[0:D, 32:64] = wk
    c32[0:D, 64:192] = np.tile(wv.T, (1, 4))
    blk = np.arange(128) // 32
    c16 = (blk[:, None] == blk[None, :]).astype(np.float16)
    return c32, c16


def make_xt(x16):
    """[n, 8192, 32] fp16 -> [n, 4, 32, 2048] with
    xt[b, j, k, e] = x[b, 2048 j + e, k]."""
    n = x16.shape[0]
    return np.ascontiguousarray(
        x16.reshape(n, 4, EB, D).transpose(0, 1, 3, 2)
    )


def make_xn(x16):
    """[n, 8192, 32] fp16 -> [n, 128, 2048] with
    xn[b, p, (t, j2, k)] = x[b, 2048 j2 + 128 t + p, k]."""
    n = x16.shape[0]
    # s = 2048 j2 + 128 t + p  ->  reshape to [n, j2, t, p, k], then
    # permute to [n, p, t, j2, k]
    return np.ascontiguousarray(
        x16.reshape(n, 4, 16, 128, D).transpose(0, 3, 2, 1, 4)
    ).reshape(n, 128, 2048)


def unpack_out(res_out):
    """[n, 4, 32, 2048] fp16 -> [n, 32, 8192] fp32."""
    return np.ascontiguousarray(
        res_out.astype(np.float32).transpose(0, 2, 1, 3)
    ).reshape(res_out.shape[0], D, S)


def make_in_maps(x, wq, wk, wv):
    c32, c16 = make_consts(wq, wk, wv)
    x16 = x.astype(np.float16)
    return [
        {
            "xt": make_xt(x16[c * PER_CORE:(c + 1) * PER_CORE]),
            "xn": make_xn(x16[c * PER_CORE:(c + 1) * PER_CORE]),
            "c32": c32,
            "c16": c16,
        }
        for c in range(N_CORES)
    ]


def kernel(x, Wq, Wk, Wv):
    x = np.asarray(x, dtype=np.float32)
    wq = np.asarray(Wq, dtype=np.float32).reshape(D, D)
    wk = np.asarray(Wk, dtype=np.float32).reshape(D, D)
    wv = np.asarray(Wv, dtype=np.float32).reshape(D, D)
    assert x.shape == (B, S, D)

    nc = _get_nc()
    in_maps = make_in_maps(x, wq, wk, wv)
    res = run_bass_kernel_spmd(nc, in_maps, list(range(N_CORES)))
    out = np.concatenate(
        [unpack_out(res.results[c]["out"]) for c in range(N_CORES)], axis=0
    )
    return out


# revision 17
# speedup vs baseline: 2.0592x; 1.1685x over previous
"""Trainium2 Bass kernel for nn_Attention_49185965473844.

Math (per example b):
    q = x @ Wq ; k = x @ Wk ; v = x @ Wv          (x: [S, D], W*: [D, D], D=32)
    A[q,k]   = sum_s q[s,q] k[s,k]  = (Wq^T G Wk)[q,k],   G = x^T x   ([32, 32])
    scores   = softmax(A, axis=q)                 (normalize down columns)
    out[q,s] = sum_k scores[q,k] v[s,k] = (M @ x^T)[q,s], M = scores @ Wv^T

The kernel is DMA- and PE-bound.  Three levers vs. the fp32 baseline:
  1. fp16 end-to-end (10-bit mantissa, same as the f32r/TF32 PE mode the
     fp32 baseline used).  Measured end-to-end rel err ~8e-4 (limit 2e-2).
  2. Host-side pre-permutation of x into BOTH layouts the PE needs:
       xt[b,j,k,e]    = x[b, 2048 j + e, k]   (x^T tiles: output-matmul rhs)
       xn[b,p,t,j2,k] = x[b, 2048 j2 + 128 t + p, k]  (natural tiles: Gram)
     so there are NO on-chip transposes at all, and every DMA (loads and
     stores) is fully contiguous in HBM with 4 KiB per partition line.
  3. The tiny 32x32 chain runs on ACT/DVE with the j2-block fold done by
     partition-preserving copies + a 128-partition contraction (the PE
     cannot matmul at base partition 96).

Per example: 16 Gram matmuls (128x128 fp16, FWL weight loads), the 32x32
chain + softmax, 4 block-diagonal output matmuls [128,512] against the
resident x^T tile, one contiguous 512 KiB store ([b, j, q, e] fp16,
un-permuted and cast to fp32 on the host).

Sharding: pure data parallel over batch B=64 -> 8 examples per NeuronCore.
"""

import numpy as np

import concourse.bass as bass
import concourse.bacc as bacc
import concourse.tile as tile
from concourse import mybir
from concourse.bass_utils import run_bass_kernel_spmd

N_CORES = 8
B, S, D = 64, 8192, 32
PER_CORE = B // N_CORES  # 8

F32 = mybir.dt.float32
F16 = mybir.dt.float16

EB = S // 4  # 2048: e-range per j-block of the transposed layout


def build_nc(n_ex=PER_CORE, seq=S):
    """Build the per-core Bass program. Same program runs on all 8 cores."""
    assert seq == S
    n_slab = seq // 512  # 16 Gram slabs per example

    nc = bacc.Bacc("TRN2", target_bir_lowering=False, debug=False)
    xt_t = nc.declare_dram_parameter("xt", [n_ex, 4, D, EB], F16, isOutput=False)
    xn_t = nc.declare_dram_parameter("xn", [n_ex, 128, 2048], F16, isOutput=False)
    c32_t = nc.declare_dram_parameter("c32", [128, 192], F32, isOutput=False)
    c16_t = nc.declare_dram_parameter("c16", [128, 128], F16, isOutput=False)
    out_t = nc.declare_dram_parameter("out", [n_ex, 4, D, EB], F16, isOutput=True)

    with tile.TileContext(nc) as tc:
        with (
            tc.tile_pool(name="consts", bufs=1) as consts,
            tc.tile_pool(name="xt_pool", bufs=n_ex) as xt_pool,
            tc.tile_pool(name="xn_pool", bufs=n_ex) as xn_pool,
            tc.tile_pool(name="osb_pool", bufs=3) as osb_pool,
            tc.tile_pool(name="small_pool", bufs=3) as small_pool,
            tc.tile_pool(name="acc_psum", bufs=4, space="PSUM") as acc_psum,
            tc.tile_pool(name="o_psum", bufs=2, space="PSUM") as o_psum,
        ):
            # ---- constants: one fp32 DMA + one fp16 DMA ----
            c32 = consts.tile([128, 192], F32)
            nc.sync.dma_start(out=c32, in_=c32_t[:, :])
            wq4 = c32[:, 0:32]           # np.tile(Wq, (4, 1))
            wk_sb = c32[0:D, 32:64]
            wvt_rep = c32[0:D, 64:192]   # wvt_rep[k, 32j + d] = Wv[d, k]
            c16 = consts.tile([128, 128], F16)
            nc.sync.dma_start(out=c16, in_=c16_t[:, :])
            blkmask = c16[:, 0:128]      # [p, c] = 1.0 iff p//32 == c//32

            def load_pair(b):
                xn = xn_pool.tile([128, 2048], F16, tag="xn", name=f"xn_{b}")
                nc.sync.dma_start(out=xn, in_=xn_t[b])
                xt = xt_pool.tile([128, EB], F16, tag="xt", name=f"xt_{b}")
                nc.sync.dma_start(
                    out=xt, in_=xt_t[b].rearrange("j k e -> (j k) e")
                )
                return xn, xt

            def emit_gram(b, xn):
                """16 Gram matmuls + column-align the 4 diagonal 32x32
                blocks with partition-preserving copies -> gram_c."""
                gram_ps = acc_psum.tile([128, 128], F32, tag="acc")
                for t in range(n_slab):
                    nc.tensor.matmul(
                        gram_ps,
                        lhsT=xn[:, 128 * t:128 * (t + 1)],
                        rhs=xn[:, 128 * t:128 * (t + 1)],
                        start=(t == 0),
                        stop=(t == n_slab - 1),
                    )
                gram_c = small_pool.tile([128, D], F32, tag="gram_c")
                for j2 in range(4):
                    if j2 % 2 == 0:
                        nc.scalar.copy(
                            out=gram_c[32 * j2:32 * (j2 + 1), :],
                            in_=gram_ps[32 * j2:32 * (j2 + 1),
                                        32 * j2:32 * (j2 + 1)])
                    else:
                        nc.vector.tensor_copy(
                            out=gram_c[32 * j2:32 * (j2 + 1), :],
                            in_=gram_ps[32 * j2:32 * (j2 + 1),
                                        32 * j2:32 * (j2 + 1)])
                return gram_c

            def emit_chain(b, gram_c):
                """t2 = G Wq (fold over j2 + multiply), A^T = Wk^T t2,
                softmax over q (free dim of A^T) -> sc_sb."""
                t2_ps = acc_psum.tile([D, D], F32, tag="acc")
                nc.tensor.matmul(t2_ps, lhsT=gram_c, rhs=wq4)
                t2_sb = small_pool.tile([D, D], F32, tag="t2_sb")
                nc.scalar.copy(out=t2_sb, in_=t2_ps)
                at_ps = acc_psum.tile([D, D], F32, tag="acc")
                nc.tensor.matmul(at_ps, lhsT=wk_sb, rhs=t2_sb)

                nmax = small_pool.tile([D, 1], F32, tag="nmax")
                nc.vector.reduce_max(
                    out=nmax, in_=at_ps, axis=mybir.AxisListType.X, negate=True
                )
                e_sb = small_pool.tile([D, D], F32, tag="e_sb")
                rsum = small_pool.tile([D, 1], F32, tag="rsum")
                nc.scalar.activation(
                    out=e_sb, in_=at_ps,
                    func=mybir.ActivationFunctionType.Exp,
                    bias=nmax, scale=1.0,
                    accum_out=rsum,
                )
                rinv = small_pool.tile([D, 1], F32, tag="rinv")
                nc.vector.reciprocal(out=rinv, in_=rsum)
                sc_sb = small_pool.tile([D, D], F32, tag="sc_sb")
                nc.vector.tensor_scalar_mul(out=sc_sb, in0=e_sb, scalar1=rinv)
                return sc_sb

            def emit_m4bd(b, sc_sb):
                """M^T replicated on 4 partition blocks -> block-diag fp16."""
                m4_ps = acc_psum.tile([128, D], F32, tag="acc")
                nc.tensor.matmul(m4_ps, lhsT=wvt_rep, rhs=sc_sb)
                m4_sb = small_pool.tile([128, D], F16, tag="m4_sb")
                nc.scalar.copy(out=m4_sb, in_=m4_ps)
                bd = small_pool.tile([128, 128], F16, tag="bd")
                m4_bcast = bass.AP(
                    tensor=m4_sb.tensor,
                    offset=m4_sb.offset,
                    ap=[list(m4_sb.ap[0]), [0, 4], list(m4_sb.ap[1])],
                )
                nc.gpsimd.tensor_mul(
                    out=bd.rearrange("p (r q) -> p r q", r=4),
                    in0=m4_bcast,
                    in1=blkmask.rearrange("p (r q) -> p r q", r=4),
                )
                return bd

            def emit_out(b, xt, bd):
                """4 block-diagonal matmuls against resident x^T + store."""
                osb = osb_pool.tile([128, 2048], F16, tag="osb",
                                    name=f"osb_{b}")
                for h in range(2):
                    o_ps = o_psum.tile([128, 1024], F32, tag="o")
                    for g in range(2):
                        nc.tensor.matmul(
                            o_ps[:, 512 * g:512 * (g + 1)],
                            lhsT=bd,
                            rhs=xt[:, 1024 * h + 512 * g:1024 * h + 512 * (g + 1)],
                        )
                    if h == 0:
                        nc.scalar.copy(out=osb[:, 0:1024], in_=o_ps)
                    else:
                        nc.vector.tensor_copy(out=osb[:, 1024:2048], in_=o_ps)
                nc.scalar.dma_start(
                    out=out_t[b].rearrange("j q e -> (j q) e"), in_=osb
                )

            # All loads queued upfront: the sync ring streams 8 MiB of
            # contiguous reads while compute pipelines behind it.
            tiles = [load_pair(b) for b in range(n_ex)]
            # Depth-5 software pipeline: gram(b) | chain(b-1) | m4bd(b-2) |
            # out(b-3).  Every cross-engine hop (gram_c copies -> t2,
            # softmax -> m4, bd -> out) gets a full gram block of slack, so
            # the PE stream is continuous (which also lets its clock ramp).
            gcs, scs, bds = {}, {}, {}
            for b in range(n_ex):
                gcs[b] = emit_gram(b, tiles[b][0])
                if b >= 1:
                    scs[b - 1] = emit_chain(b - 1, gcs.pop(b - 1))
                if b >= 2:
                    bds[b - 2] = emit_m4bd(b - 2, scs.pop(b - 2))
                if b >= 3:
                    emit_out(b - 3, tiles[b - 3][1], bds.pop(b - 3))
            n = n_ex
            scs[n - 1] = emit_chain(n - 1, gcs.pop(n - 1))
            bds[n - 2] = emit_m4bd(n - 2, scs.pop(n - 2))
            emit_out(n - 3, tiles[n - 3][1], bds.pop(n - 3))
            bds[n - 1] = emit_m4bd(n - 1, scs.pop(n - 1))
            emit_out(n - 2, tiles[n - 2][1], bds.pop(n - 2))
            emit_out(n - 1, tiles[n - 1][1], bds.pop(n - 1))

    nc.compile()
    return nc


_CACHED_NC = None


def _get_nc():
    global _CACHED_NC
    if _CACHED_NC is None:
        _CACHED_NC = build_nc()
    return _CACHED_NC


def make_consts(wq, wk, wv):
    """c32 [128, 192] fp32: Wq tiled 4x down | Wk | Wv^T tiled 4x along cols.
    c16 [128, 128] fp16: 32x32 block mask."""
    c32 = np.zeros((128, 192), dtype=np.float32)
    c32[:, 0:32] = np.tile(wq, (4, 1))
    c32[0:D, 32:64] = wk
    c32[0:D, 64:192] = np.tile(wv.T, (1, 4))
    blk = np.arange(128) // 32
    c16 = (blk[:, None] == blk[None, :]).astype(np.float16)
    return c32, c16


def make_xt(x16):
    """[n, 8192, 32] fp16 -> [n, 4, 32, 2048] with
    xt[b, j, k, e] = x[b, 2048 j + e, k]."""
    n = x16.shape[0]
    return np.ascontiguousarray(
        x16.reshape(n, 4, EB, D).transpose(0, 1, 3, 2)
    )


def make_xn(x16):
    """[n, 8192, 32] fp16 -> [n, 128, 2048] with
    xn[b, p, (t, j2, k)] = x[b, 2048 j2 + 128 t + p, k]."""
    n = x16.shape[0]
    # s = 2048 j2 + 128 t + p  ->  reshape to [n, j2, t, p, k], then
    # permute to [n, p, t, j2, k]
    return np.ascontiguousarray(
        x16.reshape(n, 4, 16, 128, D).transpose(0, 3, 2, 1, 4)
    ).reshape(n, 128, 2048)


def unpack_out(res_out):
    """[n, 4, 32, 2048] fp16 -> [n, 32, 8192] fp32."""
    return np.ascontiguousarray(
        res_out.astype(np.float32).transpose(0, 2, 1, 3)
    ).reshape(res_out.shape[0], D, S)


def make_in_maps(x, wq, wk, wv):
    c32, c16 = make_consts(wq, wk, wv)
    x16 = x.astype(np.float16)
    return [
        {
            "xt": make_xt(x16[c * PER_CORE:(c + 1) * PER_CORE]),
            "xn": make_xn(x16[c * PER_CORE:(c + 1) * PER_CORE]),
            "c32": c32,
            "c16": c16,
        }
        for c in range(N_CORES)
    ]


def kernel(x, Wq, Wk, Wv):
    x = np.asarray(x, dtype=np.float32)
    wq = np.asarray(Wq, dtype=np.float32).reshape(D, D)
    wk = np.asarray(Wk, dtype=np.float32).reshape(D, D)
    wv = np.asarray(Wv, dtype=np.float32).reshape(D, D)
    assert x.shape == (B, S, D)

    nc = _get_nc()
    in_maps = make_in_maps(x, wq, wk, wv)
    res = run_bass_kernel_spmd(nc, in_maps, list(range(N_CORES)))
    out = np.concatenate(
        [unpack_out(res.results[c]["out"]) for c in range(N_CORES)], axis=0
    )
    return out
